# revision 2
# baseline (speedup 1.0000x reference)
"""Multi-head attention (B=2, S=4096, D=768, H=12) on 8 trn2 NeuronCores, v2.

Sharding: data-parallel over batch (2) x tensor-parallel over head groups (4):
core c -> batch c//4, heads [3*(c%4), 3*(c%4)+3).

v2 design (vs v1 baseline at ~800-900us):
- Inputs are transposed on the HOST: x^T [768, 4096] f16 arrives DMA-ready for
  the projection matmuls, killing all 576 PE transposes and the PSUM->SBUF
  strip copies of v1.
- Phase A projects Q/K/V for all of S up front (PE otherwise idles while ACT
  is the attention bottleneck anyway); weight/activation casts ride the
  otherwise-idle ACT engine.
- Phase B: per 512-qpos section, heads sequential, per (head, ktile) unit:
  one f16 scores matmul [128k, 512q] -> exp -> one f16 attn@V matmul
  accumulating [65, 512] (ones column provides the softmax denominator).
  psS singles (1 PSUM bank each, bufs=4) give the PE a 3-unit lookahead, so
  the exp latency (~700ns) hides entirely behind ~1.3us of queued PE work and
  the tensor queue never stalls -> the PE stays continuously busy and ramps
  to its full 2.4GHz p-state (v1 sat at 1.2GHz forever because it stalled on
  exp every ktile).
- exp is split across TWO engines: ACT does ~70% (exact table exp), the
  vector engine does ~30% via a Schraudolph bit-trick (one tensor_scalar:
  i16 = round(s*184.665 + 15315.5), bits reinterpreted as f16 ~= exp(s/8),
  max rel err ~3% on those tiles, which softmax-averages to ~1e-2 absmax on
  the final output -- inside the 2e-2 gate). This breaks the ACT-only exp
  floor (~400us) that would otherwise bound the kernel.
- softmax denominators: reciprocal_approx_fast (5x faster than the v1
  reciprocal), gpsimd partition_broadcast, one fused scalar_tensor_tensor
  per head to write normalized W_o stationaries.
- One f16 ReduceScatter per section (8 total) keeps the collective stream
  busy all kernel instead of v1's 5 chunky ones, and shrinks the exposed
  tail collective.
"""
import contextlib
import ctypes
import sys
import types

import numpy as np

# ---------------------------------------------------------------------------
# NTFF profile hook (image's antenv lacks axon_hooks; install shim so
# run_bass_kernel_spmd(trace=True) can capture exec_time_ns).
# ---------------------------------------------------------------------------
def _install_ntff_hook():
    try:
        from antenv.axon_hooks import get_axon_ntff_profile_hook  # noqa: F401
        return
    except ImportError:
        pass
    import antenv

    mod = types.ModuleType("antenv.axon_hooks")
    _state = {"hook": None}
    mod.set_axon_ntff_profile_hook = lambda h: _state.__setitem__("hook", h)
    mod.get_axon_ntff_profile_hook = lambda: _state["hook"]
    sys.modules["antenv.axon_hooks"] = mod
    antenv.axon_hooks = mod

    try:
        lib = ctypes.CDLL("/opt/axon/libaxon_pjrt.so")
    except OSError:
        return
    if not hasattr(lib, "axon_start_nrt_profile"):
        return
    lib.axon_start_nrt_profile.argtypes = [ctypes.POINTER(ctypes.c_int64), ctypes.c_size_t]
    lib.axon_start_nrt_profile.restype = ctypes.c_int64
    lib.axon_stop_nrt_profile.argtypes = [ctypes.c_char_p]
    lib.axon_stop_nrt_profile.restype = ctypes.c_int64

    @contextlib.contextmanager
    def _hook(output_dir, device_ids):
        import jax

        jax.devices()
        if device_ids:
            ids = (ctypes.c_int64 * len(device_ids))(*device_ids)
            rc = lib.axon_start_nrt_profile(ids, len(device_ids))
        else:
            rc = lib.axon_start_nrt_profile(None, 0)
        if rc != 0:
            raise RuntimeError(f"axon_start_nrt_profile rc={rc}")
        try:
            yield
        finally:
            n = lib.axon_stop_nrt_profile(str(output_dir).encode())
            print(f"ntff profile: {n} file(s) -> {output_dir}", file=sys.stderr)

    mod.set_axon_ntff_profile_hook(_hook)


_install_ntff_hook()

import concourse.bass as bass  # noqa: E402
import concourse.tile as tile  # noqa: E402
from concourse import bacc, bass_utils, mybir  # noqa: E402

f32 = mybir.dt.float32
f16 = mybir.dt.float16
i16 = mybir.dt.int16
AF = mybir.ActivationFunctionType
ALU = mybir.AluOpType

B, S, D = 2, 4096, 768
H, DH = 12, 64
NCORES = 8
HPC = 3                 # heads per core
E = HPC * DH            # 192: per-core projection width
NSEC = 8                # q sections
QSEC = S // NSEC        # 512
NST = S // 128          # 32 k-tiles
NBLK = 8                # phase-A s-blocks of 512
NDC = 6                 # d-chunks of 128

# Schraudolph f16 exp: i16 = round(s_raw*SCHR_A + SCHR_B); bits-as-f16 ~ exp(s_raw/8)
SCHR_A = 0.125 * 1024.0 / float(np.log(2.0))   # 184.6649652...
SCHR_B = 15315.5
# k-tiles (of 32 per head) whose exp runs on the vector engine via Schraudolph
N_DVE_TILES = 12
DVE_TILES = set(int(t) for t in np.round(np.linspace(1, 30, N_DVE_TILES))) if N_DVE_TILES else set()

RG = [[0, 1, 2, 3], [4, 5, 6, 7]]
# ReduceScatter grouping over q-sections: bigger collectives amortize the
# ~25us fixed collective latency; the final one stays small (exposed tail).
RS_GROUPS = [(0, 1), (2, 3), (4, 5), (6,), (7,)]
DEBUG = False


def _build_nc():
    nc = bacc.Bacc("TRN2", target_bir_lowering=False, debug=False, num_devices=NCORES)
    xqT = nc.dram_tensor("xqT", [D, S], f16, kind="ExternalInput").ap()
    xkT = nc.dram_tensor("xkT", [D, S], f16, kind="ExternalInput").ap()
    xvT = nc.dram_tensor("xvT", [D, S], f16, kind="ExternalInput").ap()
    wqT = nc.dram_tensor("wqT", [D, E], f16, kind="ExternalInput").ap()
    wkT = nc.dram_tensor("wkT", [D, E], f16, kind="ExternalInput").ap()
    wvT = nc.dram_tensor("wvT", [D, E], f16, kind="ExternalInput").ap()
    woT = nc.dram_tensor("woT", [E, D], f16, kind="ExternalInput").ap()
    y = nc.dram_tensor("y", [NSEC * 128, D], f16, kind="ExternalOutput").ap()
    dbg_po = nc.dram_tensor("dbg_po", [NSEC * HPC * 65, 512], f32, kind="ExternalOutput").ap() if DEBUG else None
    dbg_ot = nc.dram_tensor("dbg_ot", [NSEC * 192, 512], f16, kind="ExternalOutput").ap() if DEBUG else None
    dbg_ys = nc.dram_tensor("dbg_ys", [NSEC * 512, D], f16, kind="ExternalOutput").ap() if DEBUG else None
    dbg_von = nc.dram_tensor("dbg_von", [128, NST * HPC * 65], f16, kind="ExternalOutput").ap() if DEBUG else None
    dbg_qk = nc.dram_tensor("dbg_qk", [128 + 128, S], f16, kind="ExternalOutput").ap() if DEBUG else None

    with tile.TileContext(nc) as tc:
        _body(tc, xqT, xkT, xvT, wqT, wkT, wvT, woT, y, dbg_po, dbg_ot, dbg_ys, dbg_von, dbg_qk)
    nc.compile()
    return nc


def _body(tc, xqT, xkT, xvT, wqT, wkT, wvT, woT, y, dbg_po=None, dbg_ot=None, dbg_ys=None, dbg_von=None, dbg_qk=None):
    nc = tc.nc
    with contextlib.ExitStack() as ctx:
        big = ctx.enter_context(tc.tile_pool(name="big", bufs=1))
        xload_p = ctx.enter_context(tc.tile_pool(name="xload", bufs=24))
        et_p = ctx.enter_context(tc.tile_pool(name="et", bufs=8))
        nrm_p = ctx.enter_context(tc.tile_pool(name="nrm", bufs=4))
        ot_p = ctx.enter_context(tc.tile_pool(name="ot", bufs=2))
        ys_p = ctx.enter_context(tc.tile_pool(name="ys", bufs=4))
        # PSUM budget (8 banks of 2KB/partition):
        #   psSp: score pairs [128,1024] f32 x2   = 4 banks
        #   psSs: score singles [128,512] f32 x1  = 1 bank
        #   po:   attn out [65,512] f32 x2        = 2 banks
        #   py:   W_o / Q-proj staging x1         = 1 bank
        ps_p = ctx.enter_context(tc.tile_pool(name="ps_p", bufs=2, space="PSUM"))
        ps_s = ctx.enter_context(tc.tile_pool(name="ps_s", bufs=1, space="PSUM"))
        ps_o = ctx.enter_context(tc.tile_pool(name="ps_o", bufs=2, space="PSUM"))
        ps_w = ctx.enter_context(tc.tile_pool(name="ps_w", bufs=1, space="PSUM"))
        dram = ctx.enter_context(tc.tile_pool(name="dram", bufs=1, space="DRAM"))

        # ---- persistent SBUF ----
        wq_sb = big.tile([128, NDC * E], f16)
        wk_sb = big.tile([128, NDC * E], f16)
        wv_sb = big.tile([128, NDC * E], f16)
        wo_r0 = big.tile([128, D], f16)
        wo_r1 = big.tile([64, D], f16)
        for dc in range(NDC):
            nc.gpsimd.dma_start(wk_sb[:, dc * E:(dc + 1) * E],
                                wkT[dc * 128:(dc + 1) * 128, :])

        KT0 = big.tile([128, S], f16)
        KT1 = big.tile([64, S], f16)
        VON = big.tile([128, NST * HPC * 65], f16)
        v4 = VON[:].rearrange("p (t h c) -> p t h c", h=HPC, c=65)
        nc.vector.memset(v4[:, :, :, 64:65], 1.0)
        QT0s = [big.tile([128, 512], f16, name=f"qt0_{s}") for s in range(NSEC)]
        QT1s = [big.tile([64, 512], f16, name=f"qt1_{s}") for s in range(NSEC)]

        def load_xT(x_dram, blk, eng, tag="xload"):
            xt = []
            for dc in range(NDC):
                t = xload_p.tile([128, 512], f16, tag=tag)
                eng.dma_start(t[:], x_dram[dc * 128:(dc + 1) * 128,
                                           blk * 512:(blk + 1) * 512])
                xt.append(t)
            return xt

        # ---- phase A: K and V projections for all of S ----
        # (remaining weight DMAs issued after the first x-block loads so the
        # K projection's inputs get the DMA bandwidth first)
        def late_weights():
            for dc in range(NDC):
                nc.sync.dma_start(wv_sb[:, dc * E:(dc + 1) * E],
                                  wvT[dc * 128:(dc + 1) * 128, :])
                nc.sync.dma_start(wq_sb[:, dc * E:(dc + 1) * E],
                                  wqT[dc * 128:(dc + 1) * 128, :])
            nc.sync.dma_start(wo_r0[:], woT[0:128, :])
            nc.sync.dma_start(wo_r1[:], woT[128:192, :])

        for blk in range(NBLK):
            csl = slice(blk * 512, (blk + 1) * 512)
            kt = load_xT(xkT, blk, nc.gpsimd)
            vt = load_xT(xvT, blk, nc.gpsimd)
            if blk == 0:
                late_weights()
            for lo, sz, dst in ((0, 128, KT0), (128, 64, KT1)):
                pp = ps_p.tile([128, 1024], f32, tag="psSp")
                for dc in range(NDC):
                    nc.tensor.matmul(pp[0:sz, 0:512],
                                     wk_sb[:, dc * E + lo:dc * E + lo + sz],
                                     kt[dc][:],
                                     start=(dc == 0), stop=(dc == NDC - 1))
                nc.scalar.copy(dst[0:sz, csl], pp[0:sz, 0:512])
            for st in range(4):
                t = blk * 4 + st
                pv = ps_p.tile([128, 1024], f32, tag="psSp")
                for dc in range(NDC):
                    nc.tensor.matmul(pv[:, 0:E],
                                     vt[dc][:, st * 128:(st + 1) * 128],
                                     wv_sb[:, dc * E:(dc + 1) * E],
                                     start=(dc == 0), stop=(dc == NDC - 1))
                nc.scalar.copy(v4[:, t, :, 0:64],
                               pv[:, 0:E].rearrange("p (h c) -> p h c", c=64))

        if DEBUG:
            nc.sync.dma_start(dbg_von[:], VON[:])
            nc.sync.dma_start(dbg_qk[128:256, :], KT0[:])

        # ---- Q projection: per-section, first inline, rest deferred ----
        qtiles = {}

        def q_load(sec):
            qtiles[sec] = load_xT(xqT, sec, nc.gpsimd)

        def q_proj_thunks(sec):
            # 4 PE chunks of ~3 matmuls each through the ps_w ring
            out = []
            pps = {}

            def part(lo, sz, dst, half):
                def run():
                    key = (lo,)
                    if key not in pps:
                        pps[key] = ps_w.tile([128, 512], f32, tag="py",
                                             name=f"qpp_{sec}_{lo}")
                    pp = pps[key]
                    dcs = range(0, 3) if half == 0 else range(3, NDC)
                    for dc in dcs:
                        nc.tensor.matmul(pp[0:sz, :],
                                         wq_sb[:, dc * E + lo:dc * E + lo + sz],
                                         qtiles[sec][dc][:],
                                         start=(dc == 0), stop=(dc == NDC - 1))
                    if half == 1:
                        nc.scalar.copy(dst[0:sz, :], pp[0:sz, :])
                return run

            for lo, sz, dst in ((0, 128, QT0s[sec]), (128, 64, QT1s[sec])):
                out.append(part(lo, sz, dst, 0))
                out.append(part(lo, sz, dst, 1))
            return out

        # ---- phase B ----
        # Per head: 11 ACT pairs (P) and 10 DVE singles (D) interleaved:
        # P D P D ... P.  Python-side ns bookkeeping delays each attn@V until
        # its exp result is safely ready, so the in-order PE queue never
        # stalls even at the full 2.4GHz p-state.
        ND = 8                       # Schraudolph (DVE) k-tiles per head
        NP = (NST - ND) // 2         # ACT exp pairs per head
        PAT = []
        t = 0
        acc = 0.0
        for i in range(NP):
            PAT.append(("P", t)); t += 2
            acc += ND / NP
            while acc >= 1.0 and t < NST:
                PAT.append(("D", t)); t += 1
                acc -= 1.0
        while t < NST:
            PAT.append(("D", t)); t += 1
        assert t == NST and sum(2 if k == "P" else 1 for k, _ in PAT) == NST
        AV_WIN = {"P": 1250.0, "D": 950.0}   # ns from S-end to safe AV issue

        def head_attn(sec, h, deferred):
            if h < 2:
                kt_t, q_t, hb = KT0, QT0s[sec], slice(h * 64, h * 64 + 64)
            else:
                kt_t, q_t, hb = KT1, QT1s[sec], slice(0, 64)
            po = ps_o.tile([65, 512], f32, tag="po", name=f"po_{sec}_{h}")
            est = 0.0
            pend = []          # (kind, est_S_end, [(t, et_ap), ...])
            n_av = 0
            N_AV = NST

            def emit_av(item):
                nonlocal n_av
                for tt, et_ap in item[2]:
                    nc.tensor.matmul(po[:], v4[:, tt, h, :], et_ap,
                                     start=(n_av == 0), stop=(n_av == N_AV - 1))
                    n_av += 1

            def drain():
                nonlocal est
                while pend and est - pend[0][1] >= AV_WIN[pend[0][0]]:
                    item = pend.pop(0)
                    emit_av(item)
                    est += 213.0 * len(item[2])

            for kind, t0 in PAT:
                if kind == "P":
                    psp = ps_p.tile([128, 1024], f32, tag="psSp",
                                    name=f"psp_{sec}_{h}_{t0}")
                    for j in range(2):
                        nc.tensor.matmul(psp[:, j * 512:(j + 1) * 512],
                                         kt_t[hb, (t0 + j) * 128:(t0 + j + 1) * 128],
                                         q_t[hb, :], start=True, stop=True)
                    est += 427.0
                    etp = et_p.tile([128, 1024], f16, tag="etp",
                                    name=f"etp_{sec}_{h}_{t0}")
                    nc.scalar.activation(etp[:], psp[:], AF.Exp, scale=0.125)
                    pend.append(("P", est, [(t0, etp[:, 0:512]),
                                            (t0 + 1, etp[:, 512:1024])]))
                else:
                    pss = ps_s.tile([128, 512], f32, tag="psSs",
                                    name=f"pss_{sec}_{h}_{t0}")
                    nc.tensor.matmul(pss[:], kt_t[hb, t0 * 128:(t0 + 1) * 128],
                                     q_t[hb, :], start=True, stop=True)
                    est += 213.0
                    ets = et_p.tile([128, 512], f16, tag="ets",
                                    name=f"ets_{sec}_{h}_{t0}")
                    nc.vector.tensor_scalar(ets[:].bitcast(i16), pss[:],
                                            SCHR_A, SCHR_B, ALU.mult, ALU.add)
                    pend.append(("D", est, [(t0, ets[:])]))
                drain()
                if deferred and len(pend) >= 2:
                    deferred.pop(0)()
                    est += 500.0
                    drain()
            while pend:
                emit_av(pend.pop(0))
            return po

        def head_norm(sec, h, po, ot0, ot1):
            dst = ot0[0:64, :] if h == 0 else (ot0[64:128, :] if h == 1 else ot1[0:64, :])
            dnm = nrm_p.tile([1, 512], f32, tag="dnm", name=f"dnm_{sec}_{h}")
            nc.vector.tensor_copy(dnm[:], po[64:65, :])
            rch = nrm_p.tile([1, 512], f32, tag="rch", name=f"rch_{sec}_{h}")
            nc.vector.reciprocal_approx_fast(rch[:], dnm[:])
            bc = nrm_p.tile([64, 512], f32, tag="bc", name=f"bc_{sec}_{h}")
            nc.gpsimd.partition_broadcast(bc[:], rch[:])
            nc.vector.scalar_tensor_tensor(dst, po[0:64, :], 1.0, bc[:],
                                           ALU.mult, ALU.mult)
            if DEBUG:
                pos = nrm_p.tile([65, 512], f32, tag="dbgpo", name=f"dbgpo_{sec}_{h}")
                nc.vector.tensor_copy(pos[:], po[:])
                nc.sync.dma_start(dbg_po[(sec * HPC + h) * 65:(sec * HPC + h + 1) * 65, :], pos[:])

        def make_wo_thunks(sec, ot0, ot1, rs_in, row0):
            thunks = []
            yss = {}

            def wo_part(stl, e0, esz, ceng):
                def run():
                    if stl not in yss:
                        yss[stl] = ys_p.tile([128, D], f16, tag="ysb",
                                             name=f"ys_{sec}_{stl}")
                    ys = yss[stl]
                    py = ps_w.tile([128, 512], f32, tag="py",
                                   name=f"py_{sec}_{stl}_{e0}")
                    nc.tensor.matmul(py[:, 0:esz],
                                     ot0[:, stl * 128:(stl + 1) * 128],
                                     wo_r0[:, e0:e0 + esz], start=True, stop=False)
                    nc.tensor.matmul(py[:, 0:esz],
                                     ot1[0:64, stl * 128:(stl + 1) * 128],
                                     wo_r1[:, e0:e0 + esz], start=False, stop=True)
                    if ceng is nc.scalar:
                        ceng.copy(ys[:, e0:e0 + esz], py[:, 0:esz])
                    else:
                        ceng.tensor_copy(ys[:, e0:e0 + esz], py[:, 0:esz])
                    if e0 != 0:
                        r0 = row0 + stl * 128
                        nc.sync.dma_start(rs_in[r0:r0 + 128, :], ys[:])
                        if DEBUG:
                            nc.sync.dma_start(
                                dbg_ys[sec * 512 + stl * 128:
                                       sec * 512 + (stl + 1) * 128, :], ys[:])
                return run

            for stl in range(4):
                thunks.append(wo_part(stl, 0, 512, nc.vector))
                thunks.append(wo_part(stl, 512, 256, nc.scalar))
            return thunks

        def make_rs_thunk(gi, secs, rs_in):
            def run():
                rs_out = dram.tile([len(secs) * 128, D], f16, tag="rso",
                                   name=f"rso_{gi}")
                nc.gpsimd.collective_compute(
                    "ReduceScatter",
                    mybir.AluOpType.add,
                    replica_groups=RG,
                    ins=[rs_in.opt()],
                    outs=[rs_out.opt()],
                )
                y0 = secs[0] * 128
                nc.sync.dma_start(y[y0:y0 + len(secs) * 128, :], rs_out[:])
            return run

        q_load(0)
        q_load(1)
        for th in q_proj_thunks(0):
            th()
        deferred = []
        for gi, secs in enumerate(RS_GROUPS):
            rows = len(secs) * 512
            rs_in = dram.tile([rows, D], f16, tag="rsi", name=f"rsi_{gi}")
            for j, sec in enumerate(secs):
                if sec + 2 < NSEC:
                    q_load(sec + 2)
                if sec + 1 < NSEC:
                    deferred.extend(q_proj_thunks(sec + 1))
                ot0 = ot_p.tile([128, 512], f16, tag="ot0", name=f"ot0_{sec}")
                ot1 = ot_p.tile([64, 512], f16, tag="ot1", name=f"ot1_{sec}")
                for h in range(HPC):
                    po = head_attn(sec, h, deferred)
                    head_norm(sec, h, po, ot0, ot1)
                if DEBUG:
                    nc.sync.dma_start(dbg_ot[sec * 192:sec * 192 + 128, :], ot0[:])
                    nc.sync.dma_start(dbg_ot[sec * 192 + 128:sec * 192 + 192, :], ot1[:])
                deferred.extend(make_wo_thunks(sec, ot0, ot1, rs_in, j * 512))
                if sec == secs[-1]:
                    deferred.append(make_rs_thunk(gi, secs, rs_in))
        for th in deferred:
            th()


_NC_CACHE = None


def _get_nc():
    global _NC_CACHE
    if _NC_CACHE is None:
        _NC_CACHE = _build_nc()
    return _NC_CACHE


def _make_in_maps(query, key, value, W_q, W_k, W_v, W_o):
    query = np.asarray(query, dtype=np.float32)
    key = np.asarray(key, dtype=np.float32)
    value = np.asarray(value, dtype=np.float32)
    wq_t = np.asarray(W_q, np.float32).T  # [d_in, e_out]
    wk_t = np.asarray(W_k, np.float32).T
    wv_t = np.asarray(W_v, np.float32).T
    wo_t = np.asarray(W_o, np.float32).T  # [d_in(=head dims), e_out]
    xT = {}
    for b in range(B):
        xT[("q", b)] = np.ascontiguousarray(query[b].T).astype(np.float16)
        xT[("k", b)] = np.ascontiguousarray(key[b].T).astype(np.float16)
        xT[("v", b)] = np.ascontiguousarray(value[b].T).astype(np.float16)
    in_maps = []
    for c in range(NCORES):
        b, g = c // 4, c % 4
        sl = slice(g * E, (g + 1) * E)
        in_maps.append({
            "xqT": xT[("q", b)],
            "xkT": xT[("k", b)],
            "xvT": xT[("v", b)],
            "wqT": np.ascontiguousarray(wq_t[:, sl]).astype(np.float16),
            "wkT": np.ascontiguousarray(wk_t[:, sl]).astype(np.float16),
            "wvT": np.ascontiguousarray(wv_t[:, sl]).astype(np.float16),
            "woT": np.ascontiguousarray(wo_t[sl, :]).astype(np.float16),
        })
    return in_maps


def run(in_maps, trace=False):
    nc = _get_nc()
    return bass_utils.run_bass_kernel_spmd(
        nc, in_maps, core_ids=list(range(NCORES)), trace=trace)


def assemble(results):
    # Grouped ReduceScatter: a group of n sections forms an n*512-row block;
    # core g of each 4-core replica group holds rows [g*n*128, (g+1)*n*128)
    # of that block, stored at y[secs[0]*128 : secs[0]*128 + n*128].
    out = np.empty((B, S, D), np.float32)
    for c in range(NCORES):
        b, g = c // 4, c % 4
        yc = results[c]["y"]
        for secs in RS_GROUPS:
            n = len(secs)
            g0 = secs[0] * QSEC + g * n * 128
            y0 = secs[0] * 128
            out[b, g0:g0 + n * 128] = yc[y0:y0 + n * 128].astype(np.float32)
    return out


def kernel(**inputs):
    in_maps = _make_in_maps(**inputs)
    res = run(in_maps)
    return assemble(res.results)
